# revision 29
# baseline (speedup 1.0000x reference)
"""Trainium2 Bass kernel for AnchorMambaPoolingBlockGated.

Reference computation (per batch element b, channel-first x of shape (D, L)):
    xb = x.reshape(D, N, 2)                    # stride-2 blocks
    mu = xb.mean(-1); mx = xb.max(-1)          # (D, N)
    g  = sigmoid(W @ [mu; mx] + b)             # 1x1 conv over channels
    anchors = g*mx + (1-g)*mu
    out[3k]   = anchors[:, k]
    out[3k+1] = x[:, 2k]
    out[3k+2] = x[:, 2k+1]                     # out is (3N, D)  (transposed!)

Algebra used on device (verified against the reference in numpy):
    su = e + o            (e = even tokens, o = odd tokens)
    d2 = |e - o|
    z  = W1 @ su + W2 @ d2 + b      with  W1 = 0.5*(Wmu + Wmx), W2 = 0.5*Wmx
    g  = sigmoid(z)
    anchors = 0.5*(su + g*d2)

Design (v2):
  - Host stages x de-interleaved as even/odd f16 arrays -> all DVE reads are
    packed (2x mode); input HBM traffic halves to 8 MiB/core.
  - Output DRAM tensor is f16 (12 MiB/core); host upcasts.  rel-err budget
    2e-2 >> f16 error.
  - Gate matmul: su-half contracted in f16, |e-o|-half in fp8e4 with
    MatmulPerfMode.DoubleRow (2x PE throughput on that half); only one
    on-chip cast tensor (d8) needed, assigned to gpsimd.
  - Full input resident in SBUF (2 big DMAs); per 128-block output group the
    transposed anchor/even/odd tiles are assembled into one [128,1536] SBUF
    tile and written with a single fully-contiguous 384 KiB DMA.
  - PSUM cannot be a DMA source, so PE-transposed tiles are copied
    PSUM->SBUF: token copies alternate DVE/gpsimd, anchor copies go on ACT
    with the *0.5 fused into the copy (activation Copy scale).

Sharding: data-parallel over batch, core i <- batch element i (B == 8).
"""

import os
import numpy as np

import concourse.bass as bass
import concourse.tile as tile
from concourse import bacc, mybir
from concourse.alu_op_type import AluOpType
from concourse.bass_utils import run_bass_kernel_spmd

B, D, L = 8, 512, 8192
S = 2
N = L // S                # 4096 pooled blocks
LC = N * (S + 1)          # 12288 output rows per batch
P = 128
DC = D // P               # 4 channel chunks
NCORES = 8

CHUNK_N = 512             # blocks per pipeline chunk
NCHUNK = N // CHUNK_N     # 8
NGRP = CHUNK_N // P       # 4 output groups (of 128 blocks) per chunk

# "dr" = fp8 DoubleRow on the |e-o| half of the gate matmul; "f16" = all-f16
MODE = os.environ.get("KERNEL_MODE", "dr")

_cache = {}


def _build(mode: str):
    f32 = mybir.dt.float32
    f16 = mybir.dt.float16
    f8 = mybir.dt.float8e4
    use_dr = mode == "dr"

    nc = bacc.Bacc("TRN2", target_bir_lowering=False, debug=False,
                   num_devices=NCORES)

    xe_ext = nc.declare_dram_parameter("xe", [D, N], f16, isOutput=False)
    xo_ext = nc.declare_dram_parameter("xo", [D, N], f16, isOutput=False)
    w1_ext = nc.declare_dram_parameter("w1", [D, D], f16, isOutput=False)
    w2_ext = nc.declare_dram_parameter("w2", [D, D], f8 if use_dr else f16,
                                       isOutput=False)
    bias_ext = nc.declare_dram_parameter("bias", [D, 1], f32, isOutput=False)
    id_ext = nc.declare_dram_parameter("ident", [P, P], f16, isOutput=False)
    out_ext = nc.declare_dram_parameter("out", [N, 3, D], f16, isOutput=True)

    with tile.TileContext(nc) as tc:
        with (
            tc.tile_pool(name="consts", bufs=1) as p_const,
            tc.tile_pool(name="xin", bufs=1) as p_xin,
            tc.tile_pool(name="pool", bufs=4) as p_pool,
            tc.tile_pool(name="gate", bufs=4) as p_gate,
            tc.tile_pool(name="mout", bufs=3) as p_m,
            tc.tile_pool(name="pst", bufs=3, space="PSUM") as p_pst,
            tc.tile_pool(name="psa", bufs=2, space="PSUM") as p_psa,
            tc.tile_pool(name="psz", bufs=3, space="PSUM") as p_psz,
        ):
            # --- constants ---------------------------------------------------
            # consts ride the scalar HWDGE ring so the sync ring starts on
            # the input pieces immediately
            w1_sb = p_const.tile([P, DC, D], f16)               # (128, 4, 512)
            nc.scalar.dma_start(w1_sb[:],
                                w1_ext.rearrange("(k p) d -> p k d", p=P))
            w2_sb = p_const.tile([P, DC, D], f8 if use_dr else f16)
            nc.scalar.dma_start(w2_sb[:],
                                w2_ext.rearrange("(k p) d -> p k d", p=P))
            ident = p_const.tile([P, P], f16)
            nc.scalar.dma_start(ident[:], id_ext[:])
            bias_sb = p_const.tile([P, DC, 1], f32)             # (128, 4, 1)
            nc.scalar.dma_start(bias_sb[:],
                                bias_ext.rearrange("(m p) o -> p m o", p=P))

            # --- resident inputs (split into 4 DMAs each to start early) ----
            xe_all = p_xin.tile([P, DC, N], f16)                # 32 KiB/part
            xo_all = p_xin.tile([P, DC, N], f16)
            xe_v = xe_ext.rearrange("(c p) l -> p c l", p=P)
            xo_v = xo_ext.rearrange("(c p) l -> p c l", p=P)
            # xe pieces on the sync HWDGE ring; xo pieces via SWDGE (gpsimd,
            # which idles until its first su at ~14us) -> inputs stream on two
            # queues and the sync ring clears earlier for output DMAs
            QN = N // 8
            for q in range(8):
                sl = slice(q * QN, (q + 1) * QN)
                nc.sync.dma_start(xe_all[:, :, sl], xe_v[:, :, sl])
                nc.gpsimd.dma_start(xo_all[:, :, sl], xo_v[:, :, sl])

            for ci in range(NCHUNK):
                n0 = ci * CHUNK_N
                nsl = slice(n0, n0 + CHUNK_N)
                xe = xe_all[:, :, nsl]                          # (128, 4, 512)
                xo = xo_all[:, :, nsl]

                # one merged output tile for the whole chunk -> ONE DMA
                M = p_m.tile([P, NGRP, 3 * D], f16, tag="m", name=f"m{ci}")

                # --- token transposes first: PE busy while pooling runs -----
                for g in range(NGRP):
                    gsl = slice(g * P, (g + 1) * P)
                    pst = p_pst.tile([P, 2 * D], f16, tag="pst",
                                     name=f"pst{ci}_{g}")
                    for dc in range(DC):
                        csl = slice(dc * P, (dc + 1) * P)
                        nc.tensor.transpose(pst[:, csl], xe[:, dc, gsl],
                                            ident[:])
                        nc.tensor.transpose(pst[:, D + dc * P:D + (dc + 1) * P],
                                            xo[:, dc, gsl], ident[:])
                    # token copies: PSUM->SBUF must be DVE or ACT (gpsimd
                    # cannot access PSUM); DVE takes 5/8, ACT 3/8
                    if (ci * NGRP + g) % 2 == 0:
                        nc.vector.tensor_copy(M[:, g, D:3 * D], pst[:])
                    else:
                        nc.scalar.copy(M[:, g, D:3 * D], pst[:])

                # --- pooling: su on gpsimd, diff on DVE, |.|->fp8 on ACT ----
                su = p_pool.tile([P, DC, CHUNK_N], f16, tag="su",
                                 name=f"su{ci}")
                t_ = p_pool.tile([P, DC, CHUNK_N], f16, tag="t",
                                 name=f"t{ci}")
                # chunk 0's su on DVE: gpsimd wakes up late and su gates the
                # first gate matmul (ramp); steady-state su stays on gpsimd
                su_eng = nc.vector if ci == 0 else nc.gpsimd
                su_eng.tensor_tensor(su[:], xe, xo, AluOpType.add)
                nc.vector.tensor_tensor(t_[:], xe, xo, AluOpType.subtract)
                d8 = p_pool.tile([P, DC, CHUNK_N], f8 if use_dr else f16,
                                 tag="d8", name=f"d8{ci}")
                nc.scalar.activation(d8[:], t_[:],
                                     mybir.ActivationFunctionType.Abs)

                # --- gate matmul + sigmoid ----------------------------------
                g_t = p_gate.tile([P, DC, CHUNK_N], f16, tag="g",
                                  name=f"g{ci}")
                for md in range(DC):
                    msl = slice(md * P, (md + 1) * P)
                    ps = p_psz.tile([P, CHUNK_N], f32, tag="psz",
                                    name=f"psz{ci}_{md}")
                    for kc in range(DC):
                        nc.tensor.matmul(ps[:], w1_sb[:, kc, msl],
                                         su[:, kc, :],
                                         start=(kc == 0), stop=False)
                    if use_dr:
                        for t2 in range(DC // 2):
                            ksl = slice(2 * t2, 2 * t2 + 2)
                            nc.tensor.matmul(
                                ps[:], w2_sb[:, ksl, msl], d8[:, ksl, :],
                                start=False, stop=(t2 == DC // 2 - 1),
                                perf_mode=mybir.MatmulPerfMode.DoubleRow)
                    else:
                        for kc in range(DC):
                            nc.tensor.matmul(ps[:], w2_sb[:, kc, msl],
                                             d8[:, kc, :],
                                             start=False, stop=(kc == DC - 1))
                    nc.scalar.activation(g_t[:, md, :], ps[:],
                                         mybir.ActivationFunctionType.Sigmoid,
                                         bias=bias_sb[:, md, :])

                # --- blend: U = su + g*d2  (anchors = 0.5*U) ----------------
                h_t = p_gate.tile([P, DC, CHUNK_N], f16, tag="h",
                                  name=f"h{ci}")
                u_t = p_gate.tile([P, DC, CHUNK_N], f16, tag="u",
                                  name=f"u{ci}")
                nc.vector.tensor_tensor(h_t[:], g_t[:], d8[:], AluOpType.mult)
                nc.vector.tensor_tensor(u_t[:], su[:], h_t[:], AluOpType.add)

                # --- anchor transposes + scaled copy ------------------------
                for g in range(NGRP):
                    gsl = slice(g * P, (g + 1) * P)
                    psa = p_psa.tile([P, D], f16, tag="psa",
                                     name=f"psa{ci}_{g}")
                    for md in range(DC):
                        nc.tensor.transpose(psa[:, md * P:(md + 1) * P],
                                            u_t[:, md, gsl], ident[:])
                    # 0.5 fused into the mandatory PSUM->SBUF copy
                    if g % 2 == 0:
                        nc.vector.tensor_scalar(M[:, g, 0:D], psa[:], 0.5,
                                                None, AluOpType.mult)
                    else:
                        nc.scalar.mul(M[:, g, 0:D], psa[:], 0.5)

                # --- single chunk output DMA (1.5 MiB, contiguous rows) -----
                dst = out_ext[n0:n0 + CHUNK_N, :, :]
                dst = dst.rearrange("(g p) r d -> p g (r d)", p=P)
                nc.sync.dma_start(dst, M[:])

    nc.compile()
    return nc


def _get_nc(mode=MODE):
    if mode not in _cache:
        _cache[mode] = _build(mode)
    return _cache[mode]


def _prep_weights(gate_w: np.ndarray, mode: str):
    gw = np.asarray(gate_w, dtype=np.float32)
    w_mu, w_mx = gw[:, :D], gw[:, D:]
    w1 = 0.5 * (w_mu + w_mx)
    w2 = 0.5 * w_mx
    w1t = np.ascontiguousarray(w1.T.astype(np.float16))      # (c, d)
    if mode == "dr":
        f8dt = mybir.dt.np(mybir.dt.float8e4)
        w2t = np.ascontiguousarray(
            np.clip(w2.T, -240.0, 240.0).astype(f8dt))
    else:
        w2t = np.ascontiguousarray(w2.T.astype(np.float16))
    return w1t, w2t


LAST_RESULTS = None


def kernel(x, gate_w, gate_b, mask):
    global LAST_RESULTS
    mode = MODE
    nc = _get_nc(mode)

    x = np.asarray(x, dtype=np.float32)
    xf = x.astype(np.float16)
    w1t, w2t = _prep_weights(gate_w, mode)
    bias = np.ascontiguousarray(np.asarray(gate_b, np.float32).reshape(D, 1))
    ident = np.eye(P, dtype=np.float16)

    in_maps = []
    for b in range(NCORES):
        xe = np.ascontiguousarray(xf[b, :, 0::2])
        xo = np.ascontiguousarray(xf[b, :, 1::2])
        in_maps.append({"xe": xe, "xo": xo, "w1": w1t, "w2": w2t,
                        "bias": bias, "ident": ident})
    res = run_bass_kernel_spmd(nc, in_maps, core_ids=list(range(NCORES)))
    LAST_RESULTS = res
    out = np.stack([res.results[i]["out"].reshape(LC, D)
                    for i in range(NCORES)])
    return out.astype(np.float32)


# revision 34
# speedup vs baseline: 1.0159x; 1.0159x over previous
"""Trainium2 Bass kernel for AnchorMambaPoolingBlockGated.

Reference computation (per batch element b, channel-first x of shape (D, L)):
    xb = x.reshape(D, N, 2)                    # stride-2 blocks
    mu = xb.mean(-1); mx = xb.max(-1)          # (D, N)
    g  = sigmoid(W @ [mu; mx] + b)             # 1x1 conv over channels
    anchors = g*mx + (1-g)*mu
    out[3k]   = anchors[:, k]
    out[3k+1] = x[:, 2k]
    out[3k+2] = x[:, 2k+1]                     # out is (3N, D)  (transposed!)

Algebra used on device (verified against the reference in numpy):
    su = e + o            (e = even tokens, o = odd tokens)
    d2 = |e - o|
    z  = W1 @ su + W2 @ d2 + b      with  W1 = 0.5*(Wmu + Wmx), W2 = 0.5*Wmx
    g  = sigmoid(z)
    anchors = 0.5*(su + g*d2)

Design (v2):
  - Host stages x de-interleaved as even/odd f16 arrays -> all DVE reads are
    packed (2x mode); input HBM traffic halves to 8 MiB/core.
  - Output DRAM tensor is f16 (12 MiB/core); host upcasts.  rel-err budget
    2e-2 >> f16 error.
  - Gate matmul: su-half contracted in f16, |e-o|-half in fp8e4 with
    MatmulPerfMode.DoubleRow (2x PE throughput on that half); only one
    on-chip cast tensor (d8) needed, assigned to gpsimd.
  - Full input resident in SBUF (2 big DMAs); per 128-block output group the
    transposed anchor/even/odd tiles are assembled into one [128,1536] SBUF
    tile and written with a single fully-contiguous 384 KiB DMA.
  - PSUM cannot be a DMA source, so PE-transposed tiles are copied
    PSUM->SBUF: token copies alternate DVE/gpsimd, anchor copies go on ACT
    with the *0.5 fused into the copy (activation Copy scale).

Sharding: data-parallel over batch, core i <- batch element i (B == 8).
"""

import os
import numpy as np

import concourse.bass as bass
import concourse.tile as tile
from concourse import bacc, mybir
from concourse.alu_op_type import AluOpType
from concourse.bass_utils import run_bass_kernel_spmd

B, D, L = 8, 512, 8192
S = 2
N = L // S                # 4096 pooled blocks
LC = N * (S + 1)          # 12288 output rows per batch
P = 128
DC = D // P               # 4 channel chunks
NCORES = 8

CHUNK_N = 512             # blocks per pipeline chunk
NCHUNK = N // CHUNK_N     # 8
NGRP = CHUNK_N // P       # 4 output groups (of 128 blocks) per chunk

# "dr" = fp8 DoubleRow on the |e-o| half of the gate matmul; "f16" = all-f16
MODE = os.environ.get("KERNEL_MODE", "dr")

_cache = {}


def _build(mode: str):
    f32 = mybir.dt.float32
    f16 = mybir.dt.float16
    f8 = mybir.dt.float8e4
    use_dr = mode == "dr"

    nc = bacc.Bacc("TRN2", target_bir_lowering=False, debug=False,
                   num_devices=NCORES)

    xe_ext = nc.declare_dram_parameter("xe", [D, N], f16, isOutput=False)
    xo_ext = nc.declare_dram_parameter("xo", [D, N], f16, isOutput=False)
    w1_ext = nc.declare_dram_parameter("w1", [D, D], f16, isOutput=False)
    w2_ext = nc.declare_dram_parameter("w2", [D, D], f8 if use_dr else f16,
                                       isOutput=False)
    bias_ext = nc.declare_dram_parameter("bias", [D, 1], f32, isOutput=False)
    id_ext = nc.declare_dram_parameter("ident", [P, P], f16, isOutput=False)
    out_ext = nc.declare_dram_parameter("out", [N, 3, D], f16, isOutput=True)

    with tile.TileContext(nc) as tc:
        with (
            tc.tile_pool(name="consts", bufs=1) as p_const,
            tc.tile_pool(name="xin", bufs=1) as p_xin,
            tc.tile_pool(name="pool", bufs=4) as p_pool,
            tc.tile_pool(name="gate", bufs=4) as p_gate,
            tc.tile_pool(name="mout", bufs=3) as p_m,
            tc.tile_pool(name="pst", bufs=3, space="PSUM") as p_pst,
            tc.tile_pool(name="psa", bufs=2, space="PSUM") as p_psa,
            tc.tile_pool(name="psz", bufs=3, space="PSUM") as p_psz,
        ):
            # --- constants ---------------------------------------------------
            # consts ride the scalar HWDGE ring so the sync ring starts on
            # the input pieces immediately
            w1_sb = p_const.tile([P, DC, D], f16)               # (128, 4, 512)
            nc.scalar.dma_start(w1_sb[:],
                                w1_ext.rearrange("(k p) d -> p k d", p=P))
            w2_sb = p_const.tile([P, DC, D], f8 if use_dr else f16)
            nc.scalar.dma_start(w2_sb[:],
                                w2_ext.rearrange("(k p) d -> p k d", p=P))
            ident = p_const.tile([P, P], f16)
            nc.scalar.dma_start(ident[:], id_ext[:])
            bias_sb = p_const.tile([P, DC, 1], f32)             # (128, 4, 1)
            nc.scalar.dma_start(bias_sb[:],
                                bias_ext.rearrange("(m p) o -> p m o", p=P))

            # --- resident inputs (split into 4 DMAs each to start early) ----
            xe_all = p_xin.tile([P, DC, N], f16)                # 32 KiB/part
            xo_all = p_xin.tile([P, DC, N], f16)
            xe_v = xe_ext.rearrange("(c p) l -> p c l", p=P)
            xo_v = xo_ext.rearrange("(c p) l -> p c l", p=P)
            # xe pieces on the sync HWDGE ring; xo pieces via SWDGE (gpsimd,
            # which idles until its first su at ~14us) -> inputs stream on two
            # queues and the sync ring clears earlier for output DMAs
            QN = N // 8
            for q in range(8):
                sl = slice(q * QN, (q + 1) * QN)
                nc.sync.dma_start(xe_all[:, :, sl], xe_v[:, :, sl])
                nc.gpsimd.dma_start(xo_all[:, :, sl], xo_v[:, :, sl])

            # small chunks at the ends: the pipeline fills/drains in half the
            # time (ramp to first output and final drain dominate the wall)
            sizes = [256, 256] + [512] * 6 + [256, 256]
            starts = [sum(sizes[:i]) for i in range(len(sizes))]
            for ci, (n0, csz) in enumerate(zip(starts, sizes)):
                ngrp = csz // P
                nsl = slice(n0, n0 + csz)
                xe = xe_all[:, :, nsl]                          # (128, 4, csz)
                xo = xo_all[:, :, nsl]

                # one merged output tile for the whole chunk -> ONE DMA
                M = p_m.tile([P, ngrp, 3 * D], f16, tag="m", name=f"m{ci}")

                # --- token transposes first: PE busy while pooling runs -----
                for g in range(ngrp):
                    gsl = slice(g * P, (g + 1) * P)
                    pst = p_pst.tile([P, 2 * D], f16, tag="pst",
                                     name=f"pst{ci}_{g}")
                    for dc in range(DC):
                        csl = slice(dc * P, (dc + 1) * P)
                        nc.tensor.transpose(pst[:, csl], xe[:, dc, gsl],
                                            ident[:])
                        nc.tensor.transpose(pst[:, D + dc * P:D + (dc + 1) * P],
                                            xo[:, dc, gsl], ident[:])
                    # token copies: PSUM->SBUF must be DVE or ACT (gpsimd
                    # cannot access PSUM); DVE takes 5/8, ACT 3/8
                    if (n0 // P + g) % 2 == 0:
                        nc.vector.tensor_copy(M[:, g, D:3 * D], pst[:])
                    else:
                        nc.scalar.copy(M[:, g, D:3 * D], pst[:])

                # --- pooling: su on gpsimd, diff on DVE, |.|->fp8 on ACT ----
                su = p_pool.tile([P, DC, csz], f16, tag="su",
                                 name=f"su{ci}")
                t_ = p_pool.tile([P, DC, csz], f16, tag="t",
                                 name=f"t{ci}")
                # chunk 0's su on DVE: gpsimd wakes up late and su gates the
                # first gate matmul (ramp); steady-state su stays on gpsimd
                su_eng = nc.vector if ci == 0 else nc.gpsimd
                su_eng.tensor_tensor(su[:], xe, xo, AluOpType.add)
                nc.vector.tensor_tensor(t_[:], xe, xo, AluOpType.subtract)
                d8 = p_pool.tile([P, DC, csz], f8 if use_dr else f16,
                                 tag="d8", name=f"d8{ci}")
                nc.scalar.activation(d8[:], t_[:],
                                     mybir.ActivationFunctionType.Abs)

                # --- gate matmul + sigmoid ----------------------------------
                g_t = p_gate.tile([P, DC, csz], f16, tag="g",
                                  name=f"g{ci}")
                for md in range(DC):
                    msl = slice(md * P, (md + 1) * P)
                    ps = p_psz.tile([P, csz], f32, tag="psz",
                                    name=f"psz{ci}_{md}")
                    for kc in range(DC):
                        nc.tensor.matmul(ps[:], w1_sb[:, kc, msl],
                                         su[:, kc, :],
                                         start=(kc == 0), stop=False)
                    if use_dr:
                        for t2 in range(DC // 2):
                            ksl = slice(2 * t2, 2 * t2 + 2)
                            nc.tensor.matmul(
                                ps[:], w2_sb[:, ksl, msl], d8[:, ksl, :],
                                start=False, stop=(t2 == DC // 2 - 1),
                                perf_mode=mybir.MatmulPerfMode.DoubleRow)
                    else:
                        for kc in range(DC):
                            nc.tensor.matmul(ps[:], w2_sb[:, kc, msl],
                                             d8[:, kc, :],
                                             start=False, stop=(kc == DC - 1))
                    nc.scalar.activation(g_t[:, md, :], ps[:],
                                         mybir.ActivationFunctionType.Sigmoid,
                                         bias=bias_sb[:, md, :])

                # --- blend: U = su + g*d2  (anchors = 0.5*U) ----------------
                h_t = p_gate.tile([P, DC, csz], f16, tag="h",
                                  name=f"h{ci}")
                u_t = p_gate.tile([P, DC, csz], f16, tag="u",
                                  name=f"u{ci}")
                nc.vector.tensor_tensor(h_t[:], g_t[:], d8[:], AluOpType.mult)
                nc.vector.tensor_tensor(u_t[:], su[:], h_t[:], AluOpType.add)

                # --- anchor transposes + scaled copy ------------------------
                for g in range(ngrp):
                    gsl = slice(g * P, (g + 1) * P)
                    psa = p_psa.tile([P, D], f16, tag="psa",
                                     name=f"psa{ci}_{g}")
                    for md in range(DC):
                        nc.tensor.transpose(psa[:, md * P:(md + 1) * P],
                                            u_t[:, md, gsl], ident[:])
                    # 0.5 fused into the mandatory PSUM->SBUF copy
                    if g % 2 == 0:
                        nc.vector.tensor_scalar(M[:, g, 0:D], psa[:], 0.5,
                                                None, AluOpType.mult)
                    else:
                        nc.scalar.mul(M[:, g, 0:D], psa[:], 0.5)

                # --- single chunk output DMA (1.5 MiB, contiguous rows) -----
                dst = out_ext[n0:n0 + csz, :, :]
                dst = dst.rearrange("(g p) r d -> p g (r d)", p=P)
                nc.sync.dma_start(dst, M[:])

    nc.compile()
    return nc


def _get_nc(mode=MODE):
    if mode not in _cache:
        _cache[mode] = _build(mode)
    return _cache[mode]


def _prep_weights(gate_w: np.ndarray, mode: str):
    gw = np.asarray(gate_w, dtype=np.float32)
    w_mu, w_mx = gw[:, :D], gw[:, D:]
    w1 = 0.5 * (w_mu + w_mx)
    w2 = 0.5 * w_mx
    w1t = np.ascontiguousarray(w1.T.astype(np.float16))      # (c, d)
    if mode == "dr":
        f8dt = mybir.dt.np(mybir.dt.float8e4)
        w2t = np.ascontiguousarray(
            np.clip(w2.T, -240.0, 240.0).astype(f8dt))
    else:
        w2t = np.ascontiguousarray(w2.T.astype(np.float16))
    return w1t, w2t


LAST_RESULTS = None


def kernel(x, gate_w, gate_b, mask):
    global LAST_RESULTS
    mode = MODE
    nc = _get_nc(mode)

    x = np.asarray(x, dtype=np.float32)
    xf = x.astype(np.float16)
    w1t, w2t = _prep_weights(gate_w, mode)
    bias = np.ascontiguousarray(np.asarray(gate_b, np.float32).reshape(D, 1))
    ident = np.eye(P, dtype=np.float16)

    in_maps = []
    for b in range(NCORES):
        xe = np.ascontiguousarray(xf[b, :, 0::2])
        xo = np.ascontiguousarray(xf[b, :, 1::2])
        in_maps.append({"xe": xe, "xo": xo, "w1": w1t, "w2": w2t,
                        "bias": bias, "ident": ident})
    res = run_bass_kernel_spmd(nc, in_maps, core_ids=list(range(NCORES)))
    LAST_RESULTS = res
    out = np.stack([res.results[i]["out"].reshape(LC, D)
                    for i in range(NCORES)])
    return out.astype(np.float32)
